# revision 28
# baseline (speedup 1.0000x reference)
"""MultiHeadAttn (B=2, L=2048, D=512, 8 heads) on 8 TRN2 cores.

Sharding: data-parallel. Core i handles batch b=i//4, query rows
(i%4)*512..+512, all 8 heads. K/V projections over the full 2048 keys are
recomputed on each core (no collectives); gather = concat on host.

Per-core math (head-major permutation perm[n*64+j]=j*8+n applied on host):
  QT[hd,i] = (Wq[perm].T).T @ qT          (512x512)
  KT[hd,j] = ((Wk[perm]/temp).T).T @ kT   (512x2048)  temp folded into Wk
  V''[j,h*65+d] = hv[j, h*64+d], V''[j,h*65+64] = 1   (ones col -> softmax den)
  S_h = KT_h^T @ QT_h -> exp -> PV accumulates [O_h | den_h] in PSUM [65,512]
  O_h *= 1/den_h ;  x = sum_h O_h^T @ Wp_h + q ;  LayerNorm(ddof=1, eps=1e-9)

Matmul datapath in bf16 (attention contributes ~0.7% of output magnitude, so
bf16 noise is diluted ~100x); residual q + LayerNorm stay fp32.
"""

import numpy as np

B, L, D = 2, 2048, 512
NH, DH = 8, 64
ROWS = 512
TEMP = float(np.sqrt(512.0))
EPS = 1e-9

TRACE = False
TRACE_KW = {}
LAST_EXEC_NS = None
LAST_RESULTS = None

_prog = {}


def _ensure_path():
    try:
        import concourse.bass  # noqa: F401
    except ImportError:
        import sys
        sys.path.insert(0, "/opt/trn_rl_repo")


def _build(debug=False):
    _ensure_path()
    import concourse.bacc as bacc
    import concourse.mybir as mybir
    import concourse.tile as tile

    fp32 = mybir.dt.float32
    bf16 = mybir.dt.bfloat16
    AF = mybir.ActivationFunctionType
    ALU = mybir.AluOpType

    nc = bacc.Bacc("TRN2", target_bir_lowering=False, debug=False,
                   enable_asserts=True, num_devices=8)

    d_qT = nc.dram_tensor("qT", [D, ROWS], bf16, kind="ExternalInput").ap()
    d_qn = nc.dram_tensor("qnat", [ROWS, D], fp32, kind="ExternalInput").ap()
    d_kT = nc.dram_tensor("kT", [D, L], bf16, kind="ExternalInput").ap()
    d_vT = nc.dram_tensor("vT", [D, L], bf16, kind="ExternalInput").ap()
    d_lq = nc.dram_tensor("lhsTq", [D, D], bf16, kind="ExternalInput").ap()
    d_lk = nc.dram_tensor("lhsTk", [D, D], bf16, kind="ExternalInput").ap()
    d_rv = nc.dram_tensor("rhsv", [D, D], bf16, kind="ExternalInput").ap()
    d_wp = nc.dram_tensor("wp", [D, D], bf16, kind="ExternalInput").ap()
    d_sc = nc.dram_tensor("scale", [D], fp32, kind="ExternalInput").ap()
    d_of = nc.dram_tensor("offset", [D], fp32, kind="ExternalInput").ap()
    d_out = nc.dram_tensor("out", [ROWS, D], fp32, kind="ExternalOutput").ap()
    if debug:
        d_dqt = nc.dram_tensor("dbg_qt", [D, ROWS], bf16, kind="ExternalOutput").ap()
        d_dkt = nc.dram_tensor("dbg_kt", [D, L], bf16, kind="ExternalOutput").ap()
        d_dv2 = nc.dram_tensor("dbg_v2", [L, NH * 65], bf16, kind="ExternalOutput").ap()
        d_don = nc.dram_tensor("dbg_on", [D, ROWS], bf16, kind="ExternalOutput").ap()
        d_dx = nc.dram_tensor("dbg_x", [ROWS, D], fp32, kind="ExternalOutput").ap()
        d_dden = nc.dram_tensor("dbg_den", [NH, ROWS], fp32, kind="ExternalOutput").ap()
        d_dbc = nc.dram_tensor("dbg_bc", [64, ROWS], fp32, kind="ExternalOutput").ap()

    from contextlib import ExitStack
    with tile.TileContext(nc) as tc, ExitStack() as ctx:
        # ---- persistent SBUF tiles (one bufs=1 pool, distinct names) ----
        P = ctx.enter_context(tc.tile_pool(name="persist", bufs=1))
        DP = ctx.enter_context(tc.tile_pool(name="dscr", bufs=1, space="DRAM"))
        bscr = [DP.tile([1, ROWS], fp32, name=f"bscr{h}") for h in range(NH)]
        A = [P.tile([128, L], bf16, name=f"A{t}") for t in range(4)]       # kT
        AV = [P.tile([128, L], bf16, name=f"AV{t}") for t in range(4)]     # vT
        Wq1 = [P.tile([128, D], bf16, name=f"Wq{t}") for t in range(4)]
        Wk1 = [P.tile([128, D], bf16, name=f"Wk{t}") for t in range(4)]
        Wv1 = [P.tile([128, D], bf16, name=f"Wv{t}") for t in range(4)]
        U = [P.tile([128, D], bf16, name=f"U{t}") for t in range(4)]       # qT
        X = [P.tile([128, D], fp32, name=f"X{t}") for t in range(4)]       # ln scratch
        QT = [P.tile([128, ROWS], bf16, name=f"QT{t}") for t in range(4)]
        KT = [P.tile([128, L], bf16, name=f"KT{t}") for t in range(4)]
        V2 = [P.tile([128, NH * 65], bf16, name=f"V2_{j}") for j in range(16)]
        qn = [P.tile([128, D], fp32, name=f"qn{t}") for t in range(4)]
        WPp = [P.tile([128, D], bf16, name=f"WPp{g}") for g in range(4)]
        ONp = [P.tile([128, D], bf16, name=f"ONp{g}") for g in range(4)]
        ONs = P.tile([64, ROWS], bf16, name="ONs")      # odd-head staging
        rden = [P.tile([128, ROWS], fp32, name=f"rden{j}") for j in range(2)]
        bcd = [P.tile([64, ROWS], fp32, name=f"bcd{j}") for j in range(2)]
        scb = P.tile([128, D], fp32, name="scb")
        ofb = P.tile([128, D], fp32, name="ofb")
        stt = [P.tile([128, 6], fp32, name=f"stt{t}") for t in range(4)]
        mv = [P.tile([128, 2], fp32, name=f"mv{t}") for t in range(4)]
        sdt = [P.tile([128, 1], fp32, name=f"sdt{t}") for t in range(4)]
        rst = [P.tile([128, 1], fp32, name=f"rst{t}") for t in range(4)]

        # ---- phase 0: all input DMAs up front ----
        for t in range(4):
            sl = slice(t * 128, (t + 1) * 128)
            nc.sync.dma_start(out=U[t], in_=d_qT[sl, :])
            nc.sync.dma_start(out=Wq1[t], in_=d_lq[sl, :])
            nc.sync.dma_start(out=A[t], in_=d_kT[sl, :])
            nc.sync.dma_start(out=Wk1[t], in_=d_lk[sl, :])
            nc.sync.dma_start(out=AV[t], in_=d_vT[sl, :])
            nc.sync.dma_start(out=Wv1[t], in_=d_rv[sl, :])
            nc.sync.dma_start(out=WPp[t], in_=d_wp[sl, :])
            nc.sync.dma_start(out=qn[t], in_=d_qn[sl, :])
        nc.sync.dma_start(out=scb, in_=d_sc.rearrange("(p f) -> p f", p=1).broadcast_to([128, D]))
        nc.sync.dma_start(out=ofb, in_=d_of.rearrange("(p f) -> p f", p=1).broadcast_to([128, D]))

        with tc.tile_pool(name="pp", bufs=4, space="PSUM") as pp:
            # ---- phase 1: Q projection -> QT [hd, 512] ----
            for t in range(4):
                pt = pp.tile([128, ROWS], fp32, name=f"qp{t}", tag="ps")
                for dm in range(4):
                    nc.tensor.matmul(pt, Wq1[dm][:, t * 128:(t + 1) * 128],
                                     U[dm], start=(dm == 0), stop=(dm == 3))
                nc.vector.tensor_copy(out=QT[t], in_=pt)

            # ---- phase 2: K projection -> KT [hd, 2048] (temp pre-folded) ----
            for t in range(4):
                pts = [pp.tile([128, 512], fp32, name=f"kp{t}_{ls}", tag="ps") for ls in range(4)]
                for dm in range(4):
                    for ls in range(4):
                        nc.tensor.matmul(pts[ls], Wk1[dm][:, t * 128:(t + 1) * 128],
                                         A[dm][:, ls * 512:(ls + 1) * 512],
                                         start=(dm == 0), stop=(dm == 3))
                for ls in range(4):
                    nc.vector.tensor_copy(out=KT[t][:, ls * 512:(ls + 1) * 512],
                                          in_=pts[ls])

            # ---- phase 3: V projection -> V'' [j, 8*65] with ones col ----
            for j in range(16):
                v3 = V2[j].rearrange("p (h c) -> p h c", h=NH)
                nc.vector.tensor_scalar(
                    out=v3[:, :, 64:65],
                    in0=scb[:, 0:8].rearrange("p (h c) -> p h c", c=1),
                    scalar1=0.0, scalar2=1.0, op0=ALU.mult, op1=ALU.add)
                pt = pp.tile([128, D], fp32, name=f"vp{j}", tag="ps")
                for dm in range(4):
                    nc.tensor.matmul(pt, AV[dm][:, j * 128:(j + 1) * 128],
                                     Wv1[dm], start=(dm == 0), stop=(dm == 3))
                nc.vector.tensor_copy(out=v3[:, :, 0:64],
                                      in_=pt.rearrange("p (h c) -> p h c", h=NH))

        if debug:
            for t in range(4):
                nc.sync.dma_start(out=d_dqt[t * 128:(t + 1) * 128, :], in_=QT[t])
                nc.sync.dma_start(out=d_dkt[t * 128:(t + 1) * 128, :], in_=KT[t])
            for j in range(16):
                nc.sync.dma_start(out=d_dv2[j * 128:(j + 1) * 128, :], in_=V2[j])

        # ---- phase 4: attention, head-groups of 2 ----
        with tc.tile_pool(name="accp", bufs=1, space="PSUM") as accp, \
             tc.tile_pool(name="wvp", bufs=2, space="PSUM") as wvp, \
             tc.tile_pool(name="esp", bufs=2) as esp:
            acc = [accp.tile([128, ROWS], fp32, name=f"acc{i}", tag=f"acc{i}")
                   for i in range(2)]
            for g in range(4):
                h0, h1 = 2 * g, 2 * g + 1
                cells = [(h, ks) for ks in range(16) for h in (h0, h1)]
                for ws in range(0, 32, 3):
                    wc = cells[ws:ws + 3]
                    n = len(wc)
                    wv = wvp.tile([128, 3, 512], fp32, name=f"wv{g}_{ws}", tag="wv")
                    for i, (h, ks) in enumerate(wc):
                        p0 = (h % 2) * 64
                        nc.tensor.matmul(
                            wv[:, i, :],
                            KT[g][p0:p0 + 64, ks * 128:(ks + 1) * 128],
                            QT[g][p0:p0 + 64, :],
                            start=True, stop=True)
                    es = esp.tile([128, 3, 512], bf16, name=f"es{g}_{ws}", tag="es")
                    nc.scalar.activation(out=es[:, 0:n, :], in_=wv[:, 0:n, :],
                                         func=AF.Exp)
                    for i, (h, ks) in enumerate(wc):
                        nc.tensor.matmul(
                            acc[h % 2][0:65, :],
                            V2[ks][:, h * 65:h * 65 + 65],
                            es[:, i, :],
                            start=(ks == 0), stop=(ks == 15))
                # drain: normalize by softmax denominator (row 64 of acc)
                for j, h in enumerate((h0, h1)):
                    nc.vector.reciprocal(out=rden[j][64:65, :], in_=acc[j][64:65, :])
                    nc.sync.dma_start(out=bscr[h], in_=rden[j][64:65, :])
                    nc.sync.dma_start(out=bcd[j],
                                      in_=bscr[h].broadcast_to([64, ROWS]))
                    if debug:
                        nc.sync.dma_start(out=d_dden[h:h + 1, :],
                                          in_=rden[j][64:65, :])
                        if g == 0 and j == 0:
                            nc.sync.dma_start(out=d_dbc, in_=bcd[j])
                    if j == 0:
                        nc.vector.tensor_tensor(out=ONp[g][0:64, :],
                                                in0=acc[j][0:64, :], in1=bcd[j],
                                                op=ALU.mult)
                    else:
                        nc.vector.tensor_tensor(out=ONs, in0=acc[j][0:64, :],
                                                in1=bcd[j], op=ALU.mult)
                        nc.sync.dma_start(out=ONp[g][64:128, :], in_=ONs)

        if debug:
            for g in range(4):
                nc.sync.dma_start(out=d_don[g * 128:(g + 1) * 128, :], in_=ONp[g])

        # ---- phase 5: out projection + residual + LayerNorm ----
        with tc.tile_pool(name="xpp", bufs=4, space="PSUM") as xpp:
            for qs in range(4):
                xt = xpp.tile([128, D], fp32, name=f"x{qs}", tag="x")
                for g in range(4):
                    nc.tensor.matmul(xt, ONp[g][:, qs * 128:(qs + 1) * 128],
                                     WPp[g], start=(g == 0), stop=(g == 3))
                nc.vector.tensor_tensor(out=X[qs], in0=xt, in1=qn[qs], op=ALU.add)
                if debug:
                    nc.sync.dma_start(out=d_dx[qs * 128:(qs + 1) * 128, :],
                                      in_=X[qs])
                nc.vector.bn_stats(out=stt[qs], in_=X[qs])
                nc.vector.bn_aggr(out=mv[qs], in_=stt[qs])
                nc.scalar.activation(out=sdt[qs], in_=mv[qs][:, 1:2], func=AF.Sqrt,
                                     scale=float(D) / float(D - 1))
                nc.vector.tensor_scalar(out=rst[qs], in0=sdt[qs], scalar1=EPS,
                                        scalar2=None, op0=ALU.add)
                nc.vector.reciprocal(out=rst[qs], in_=rst[qs])
                nc.vector.tensor_scalar(out=X[qs], in0=X[qs],
                                        scalar1=mv[qs][:, 0:1], scalar2=rst[qs],
                                        op0=ALU.subtract, op1=ALU.mult)
                nc.vector.tensor_tensor(out=X[qs], in0=X[qs], in1=scb, op=ALU.mult)
                nc.vector.tensor_tensor(out=X[qs], in0=X[qs], in1=ofb, op=ALU.add)
                nc.sync.dma_start(out=d_out[qs * 128:(qs + 1) * 128, :], in_=X[qs])

    nc.compile()
    return nc


def _get_prog():
    if "nc" not in _prog:
        _prog["nc"] = _build()
    return _prog["nc"]


def kernel(**inputs):
    global LAST_EXEC_NS, LAST_RESULTS
    _ensure_path()
    import ml_dtypes
    from concourse.bass_utils import run_bass_kernel_spmd
    bf = ml_dtypes.bfloat16

    q = np.asarray(inputs["q"], dtype=np.float32)
    k = np.asarray(inputs["k"], dtype=np.float32)
    v = np.asarray(inputs["v"], dtype=np.float32)
    Wq = np.asarray(inputs["Wq"], dtype=np.float32)
    Wk = np.asarray(inputs["Wk"], dtype=np.float32)
    Wv = np.asarray(inputs["Wv"], dtype=np.float32)
    Wp = np.asarray(inputs["Wp"], dtype=np.float32)
    scale = np.ascontiguousarray(inputs["scale"], dtype=np.float32)
    offset = np.ascontiguousarray(inputs["offset"], dtype=np.float32)

    # head-major permutation: perm[n*64+j] = j*8+n  (heads innermost in ref)
    perm = np.arange(D).reshape(DH, NH).T.ravel()
    lhsTq = np.ascontiguousarray(Wq[perm, :].T).astype(bf)
    lhsTk = np.ascontiguousarray((Wk[perm, :] / TEMP).T).astype(bf)
    rhsv = np.ascontiguousarray(Wv[perm, :].T).astype(bf)
    wp = np.ascontiguousarray(Wp[:, perm].T).astype(bf)

    in_maps = []
    for core in range(8):
        b, r0 = core // 4, (core % 4) * ROWS
        qblk = q[b, r0:r0 + ROWS, :]
        in_maps.append({
            "qT": np.ascontiguousarray(qblk.T).astype(bf),
            "qnat": np.ascontiguousarray(qblk),
            "kT": np.ascontiguousarray(k[b].T).astype(bf),
            "vT": np.ascontiguousarray(v[b].T).astype(bf),
            "lhsTq": lhsTq, "lhsTk": lhsTk, "rhsv": rhsv, "wp": wp,
            "scale": scale, "offset": offset,
        })

    nc = _get_prog()
    res = run_bass_kernel_spmd(nc, in_maps, core_ids=list(range(8)),
                               trace=TRACE, **TRACE_KW)
    LAST_EXEC_NS = res.exec_time_ns
    LAST_RESULTS = res

    out = np.empty((B, L, D), dtype=np.float32)
    for core in range(8):
        b, r0 = core // 4, (core % 4) * ROWS
        out[b, r0:r0 + ROWS, :] = res.results[core]["out"]
    return out
